# revision 1
# baseline (speedup 1.0000x reference)
"""Trainium2 Bass kernel for nn_EnhancedAttentionLayer (GAT-style masked attention).

Data-parallel over batch: B=8 batch elements -> 8 NeuronCores, one each.
Params replicated. No collectives.

Math (per batch element, all heads on one core):
  h{1,2,3} = feat @ W{1,2,3}[h]      (per-head projections)
  t1 = tanh(h1); src = t1 @ w_src; dst = t1 @ w_dst
  attn1[i,j] = leaky_relu(src[i] + dst[j], 0.2); masked softmax over j with
  masks m2 (same-clause) / m3 (adj, cross-clause).

Key factorization: exp(leaky(z)) = exp(0.2 z) * max(exp(0.8 z), 1), and
exp(0.2 src_i) cancels in the masked softmax. With
  u'[j,i] = max(exp(.8 src_i) * exp(dst_j), exp(.2 dst_j))
          = tensor_scalar(r_rep, ed_j, ed2_j, mult, max)   (ONE fused DVE op)
the attention block is p_k = m_k * u' and
  out_k[i,:] = sum_j p_k[j,i] h_k[j,:] / sum_j p_k[j,i]
(denominator via the h23 ones-column). The [N,N] stage is TWO elementwise
ops per (head, j-chunk) vs 5 in the direct form.

Other choices:
  - p/u TRANSPOSED [j, i] to feed matmul lhsT; masks host-precomputed in that
    layout (m23T = [m2 | m3] per j-chunk).
  - h2/h3 projections + gate matmul in fp8e4m3 DoubleRow (K=256, 0.5 cyc/row).
  - gate sigmoid as (1+tanh(x/2))/2 so the WHOLE kernel needs one activation
    table (exp/tanh/relu/copy) -- zero table swaps, no ACT-order constraint.
  - elu(x) = min(exp(x)-1, relu(x)) fused; emit/blend all-bf16.
  - The cost model's DMA device is serial (~17us for all inputs), so every
    DMA goes on the sync queue in exact need-order (W1 repacked per-head so
    h1 can start ~3us in); h23 runs before the gate so attention-apply
    matmuls start as early as possible.
"""

import numpy as np
import ml_dtypes

import concourse.bass as bass
import concourse.tile as tile
from concourse import bacc, mybir
from concourse.bass_utils import run_bass_kernel_spmd

F32 = mybir.dt.float32
BF16 = mybir.dt.bfloat16
FP8 = mybir.dt.float8e4
AF = mybir.ActivationFunctionType
OP = mybir.AluOpType
PM = mybir.MatmulPerfMode

B, N, D = 8, 512, 768
H, E = 8, 96
IC = N // 128
JC = N // 128
DC = D // 128
KT3 = 3
EPS = 1e-30

GATE_FP8 = False
# (h,jc) chunks whose mask-product runs on Pool (rest on DVE)
POOL_PU = [(i * 15) // (H * JC) != ((i + 1) * 15) // (H * JC)
           for i in range(H * JC)]

_CACHED = None


def build_kernel(with_bias: bool):
    nc = bacc.Bacc("TRN2", target_bir_lowering=False, debug=False, num_devices=B)

    feat_bf = nc.dram_tensor("feat_bf", [128, IC * D], BF16, kind="ExternalInput").ap()
    featT = nc.dram_tensor("featT", [128, DC * N], BF16, kind="ExternalInput").ap()
    featT8 = nc.dram_tensor("featT8", [128, DC * N], FP8, kind="ExternalInput").ap()
    m23T = nc.dram_tensor("m23T", [128, JC * 2 * N], BF16, kind="ExternalInput").ap()
    adjd3 = nc.dram_tensor("adjd3", [128, IC], F32, kind="ExternalInput").ap()
    # W1 packed per-head: [p, (h, dc, e)] so h1(h) needs only its own chunk
    W1p = nc.dram_tensor("W1p", [128, H * DC * E], BF16, kind="ExternalInput").ap()
    W23f8 = nc.dram_tensor("W23f8", [128, DC * H * 2 * E], FP8, kind="ExternalInput").ap()
    wsr = nc.dram_tensor("wsr", [96, H * 128], BF16, kind="ExternalInput").ap()
    wsd = nc.dram_tensor("wsd", [96, 16], BF16, kind="ExternalInput").ap()
    if GATE_FP8:
        Hwt8 = nc.dram_tensor("Hwt8", [128, DC * D], FP8, kind="ExternalInput").ap()
    else:
        Hwt = nc.dram_tensor("Hwt", [128, DC * D], BF16, kind="ExternalInput").ap()
    Hb = nc.dram_tensor("Hb", [1, D], BF16, kind="ExternalInput").ap()
    eye128 = nc.dram_tensor("eye128", [128, 128], BF16, kind="ExternalInput").ap()
    ones_row = nc.dram_tensor("ones_row", [1, 512], BF16, kind="ExternalInput").ap()
    if with_bias:
        b3row = nc.dram_tensor("b3row", [1, D], BF16, kind="ExternalInput").ap()
    out = nc.dram_tensor("out", [N, D], F32, kind="ExternalOutput").ap()

    with tile.TileContext(nc) as tc:
        with tc.tile_pool(name="persist", bufs=1) as P:
            adjd3_sb = P.tile([128, IC], F32, tag="adjd3_sb")
            Hb_sb = P.tile([1, D], BF16, tag="Hb_sb")
            eye_sb = P.tile([128, 128], BF16, tag="eye_sb")
            onesr_sb = P.tile([1, 512], BF16, tag="onesr_sb")
            m23_sb = P.tile([128, JC * 2 * N], BF16, tag="m23_sb")     # 8K
            h23 = P.tile([128, JC * H * 2 * 97], BF16, tag="h23")      # 12.1K
            r_rep = P.tile([128, H * N], BF16, tag="r_rep")            # 8K
            ed_sb = P.tile([128, H * JC], F32, tag="ed_sb")
            ed2_sb = P.tile([128, H * JC], F32, tag="ed2_sb")
            tgate = P.tile([128, IC * D], BF16, tag="tgate")           # 6K
            gate_bf = P.tile([128, IC * D], BF16, tag="gate_bf")       # 6K
            preS = P.tile([128, IC * D], BF16, tag="preS")             # 6K
            intra_bf = P.tile([128, IC * D], BF16, tag="intra_bf")     # 6K
            feat_sb = P.tile([128, IC * D], BF16, tag="feat_sb")       # 6K
            b3_sb = P.tile([128, D], BF16, tag="b3_sb") if with_bias else None

            h23r = h23[:].rearrange("p (jc h k eo) -> p jc h k eo",
                                    jc=JC, h=H, k=2, eo=97)

            # rounds pools allocated FIRST so they get SBUF disjoint from the
            # weights pool -- otherwise builds wait on stage-1 SBUF frees
            UP = tc.alloc_tile_pool(name="upool", bufs=3)
            PP = tc.alloc_tile_pool(name="ppool", bufs=3)
            EV = tc.alloc_tile_pool(name="evpool", bufs=3)
            FP = tc.alloc_tile_pool(name="fpool", bufs=3)

            with tc.tile_pool(name="wpool", bufs=1) as WP:
                if GATE_FP8:
                    Hwt_sb = WP.tile([128, DC * D], FP8, tag="Hwt_sb")
                else:
                    Hwt_sb = WP.tile([128, DC * D], BF16, tag="Hwt_sb")
                featT8_sb = WP.tile([128, DC * N], FP8, tag="featT8_sb")
                W23_sb = WP.tile([128, DC * H * 2 * E], FP8, tag="W23_sb")
                W1p_sb = WP.tile([128, H * DC * E], BF16, tag="W1p_sb")
                featT_sb = WP.tile([128, DC * N], BF16, tag="featT_sb")
                wsr_sb = WP.tile([96, H * 128], BF16, tag="wsr_sb")
                wsd_sb = WP.tile([96, 16], BF16, tag="wsd_sb")
                TP = tc.alloc_tile_pool(name="tpool", bufs=1)
                t1T = TP.tile([96, H * N], BF16, tag="t1T")
                h1T = TP.tile([96, H * N], BF16, tag="h1T")

                # ---- ALL DMAs on sync queue, in exact need-order ----
                nc.sync.dma_start(featT_sb[:], featT)
                HW1 = DC * E   # 576 per head
                for h in range(3):
                    nc.sync.dma_start(
                        W1p_sb[:, h * HW1:(h + 1) * HW1],
                        W1p[:, h * HW1:(h + 1) * HW1])
                nc.sync.dma_start(wsd_sb[:], wsd)
                nc.sync.dma_start(wsr_sb[:], wsr)
                nc.sync.dma_start(m23_sb[:], m23T)
                for h in range(3, H):
                    nc.sync.dma_start(
                        W1p_sb[:, h * HW1:(h + 1) * HW1],
                        W1p[:, h * HW1:(h + 1) * HW1])
                nc.sync.dma_start(featT8_sb[:], featT8)
                nc.sync.dma_start(W23_sb[:], W23f8)
                nc.sync.dma_start(Hwt_sb[:], Hwt8 if GATE_FP8 else Hwt)
                nc.sync.dma_start(Hb_sb[:], Hb)
                nc.sync.dma_start(onesr_sb[:], ones_row)
                nc.sync.dma_start(eye_sb[:], eye128)
                nc.sync.dma_start(adjd3_sb[:], adjd3)
                nc.sync.dma_start(feat_sb[:], feat_bf)

                featT8r = featT8_sb[:].rearrange("p (kt n) -> p kt n", kt=DC)
                W23r = W23_sb[:].rearrange("p (kt f) -> p kt f", kt=DC)
                if GATE_FP8:
                    Hwtr = Hwt_sb[:].rearrange("p (kt f) -> p kt f", kt=DC)

                # ------- h1 + tanh + h1T copy, scores pipelined per head ----
                def scores(h):
                    pss = PSS.tile([128, 512], F32, tag="pss")
                    nc.tensor.matmul(pss[:, :],
                                     wsr_sb[0:96, h * 128:(h + 1) * 128],
                                     t1T[0:96, h * N:(h + 1) * N],
                                     start=True, stop=True)
                    nc.scalar.activation(r_rep[:, h * N:(h + 1) * N],
                                         pss[:, :], AF.Exp, scale=0.8)
                    psd = PSD.tile([128, 4], F32, tag="psd")
                    for jc in range(JC):
                        nc.tensor.matmul(
                            psd[:, jc:jc + 1],
                            t1T[0:96, h * N + jc * 128:h * N + (jc + 1) * 128],
                            wsd_sb[0:96, 8 + h:9 + h],
                            start=True, stop=True)
                    g0 = h * JC
                    nc.scalar.activation(ed_sb[:, g0:g0 + 4], psd[:, :],
                                         AF.Exp)
                    nc.scalar.activation(ed2_sb[:, g0:g0 + 4], psd[:, :],
                                         AF.Exp, scale=0.2)

                with (
                    tc.tile_pool(name="ps1p", bufs=4, space="PSUM") as PS1,
                    tc.tile_pool(name="pssp", bufs=2, space="PSUM") as PSS,
                    tc.tile_pool(name="psdp", bufs=2, space="PSUM") as PSD,
                ):
                    prev = None
                    for h in range(H):
                        ps1 = PS1.tile([96, 512], F32, tag="ps1")
                        for dc in range(DC):
                            nc.tensor.matmul(
                                ps1[:, :],
                                W1p_sb[:, h * HW1 + dc * E:
                                       h * HW1 + (dc + 1) * E],
                                featT_sb[:, dc * N:(dc + 1) * N],
                                start=(dc == 0), stop=(dc == DC - 1))
                        nc.scalar.activation(t1T[0:96, h * N:(h + 1) * N],
                                             ps1[:, :], AF.Tanh)
                        nc.scalar.copy(h1T[0:96, h * N:(h + 1) * N],
                                       ps1[:, :])
                        if prev is not None:
                            scores(prev)
                        prev = h
                    scores(prev)

                # ------- h2|h3 projections (fp8 DoubleRow), copies ACT/Pool -
                with tc.tile_pool(name="ps23p", bufs=2, space="PSUM") as PS23:
                    for mc in range(IC):
                        ps23 = PS23.tile([128, 2048], F32, tag="ps23")
                        for g in range(4):
                            for k3 in range(KT3):
                                nc.tensor.matmul(
                                    ps23[:, g * 512:g * 512 + 384],
                                    featT8r[:, 2 * k3:2 * k3 + 2,
                                            mc * 128:(mc + 1) * 128],
                                    W23r[:, 2 * k3:2 * k3 + 2,
                                         g * 384:(g + 1) * 384],
                                    start=(k3 == 0), stop=(k3 == KT3 - 1),
                                    perf_mode=PM.DoubleRow)
                        nc.gpsimd.memset(h23r[:, mc, :, :, 96:97], 1.0)
                        for g in range(4):
                            psrc = ps23[:, g * 512:g * 512 + 384].rearrange(
                                "p (h k e) -> p h k e", h=2, k=2, e=96)
                            dst = h23r[:, mc, 2 * g:2 * g + 2, :, 0:96]
                            if g % 2 == 0:
                                nc.scalar.copy(dst, psrc)
                            else:
                                nc.vector.tensor_copy(dst, psrc)

                # ------- gate matmuls + tanh-sigmoid ------------------------
                with (
                    tc.tile_pool(name="psgp", bufs=2, space="PSUM") as PSG,
                    tc.tile_pool(name="pstp", bufs=2, space="PSUM") as PST,
                ):
                    for mc in range(IC):
                        psg = PSG.tile([128, 1024], F32, tag="psg")
                        if GATE_FP8:
                            for k3 in range(KT3):
                                lhsT = featT8r[:, 2 * k3:2 * k3 + 2,
                                               mc * 128:(mc + 1) * 128]
                                nc.tensor.matmul(
                                    psg[:, 0:512], lhsT,
                                    Hwtr[:, 2 * k3:2 * k3 + 2, 0:512],
                                    start=(k3 == 0), stop=False,
                                    perf_mode=PM.DoubleRow)
                                nc.tensor.matmul(
                                    psg[:, 512:768], lhsT,
                                    Hwtr[:, 2 * k3:2 * k3 + 2, 512:768],
                                    start=(k3 == 0), stop=False,
                                    perf_mode=PM.DoubleRow)
                        else:
                            for dc in range(DC):
                                lhsT = featT_sb[:, dc * N + mc * 128:
                                                dc * N + (mc + 1) * 128]
                                nc.tensor.matmul(psg[:, 0:512], lhsT,
                                                 Hwt_sb[:, dc * D:dc * D + 512],
                                                 start=(dc == 0), stop=False)
                                nc.tensor.matmul(
                                    psg[:, 512:768], lhsT,
                                    Hwt_sb[:, dc * D + 512:(dc + 1) * D],
                                    start=(dc == 0), stop=False)
                        nc.tensor.matmul(psg[:, 0:512], onesr_sb[0:1, 0:128],
                                         Hb_sb[0:1, 0:512],
                                         start=False, stop=True)
                        nc.tensor.matmul(psg[:, 512:768], onesr_sb[0:1, 0:128],
                                         Hb_sb[0:1, 512:768],
                                         start=False, stop=True)
                        nc.scalar.activation(tgate[:, mc * D:(mc + 1) * D],
                                             psg[:, 0:768], AF.Tanh, scale=0.5)
                    # gate = (1 + t)/2, one wide fused op
                    nc.vector.tensor_scalar(gate_bf[:], tgate[:], 0.5, 0.5,
                                            OP.mult, OP.add)

                    # ------- h1 transpose + intra (interleaved with gate) ---
                    for ic in range(IC):
                        pst = PST.tile([128, 1024], BF16, tag="pst")
                        for h in range(H):
                            nc.tensor.transpose(
                                pst[:, h * 128:h * 128 + 96],
                                h1T[0:96, h * N + ic * 128:h * N + (ic + 1) * 128],
                                eye_sb[0:96, 0:96])
                        pstr = pst[:].rearrange("p (u h o) -> p u h o",
                                                u=1, h=H, o=128)
                        nc.scalar.activation(
                            intra_bf[:, ic * D:(ic + 1) * D]
                            .rearrange("p (u h e) -> p u h e", u=1, h=H, e=96),
                            pstr[:, :, :, 0:96],
                            AF.Copy, scale=adjd3_sb[:, ic:ic + 1])
                TP.release()

                if with_bias:
                    with tc.tile_pool(name="psbp", bufs=1, space="PSUM") as PSB:
                        psb = PSB.tile([128, D], F32, tag="psb")
                        b3d = WP.tile([1, D], BF16, tag="b3d")
                        nc.sync.dma_start(b3d[:], b3row)
                        nc.tensor.matmul(psb[:, 0:512], onesr_sb[0:1, 0:128],
                                         b3d[0:1, 0:512], start=True, stop=True)
                        nc.tensor.matmul(psb[:, 512:768], onesr_sb[0:1, 0:128],
                                         b3d[0:1, 512:768], start=True, stop=True)
                        nc.vector.tensor_copy(b3_sb[:], psb[:, :])

            # ---------------- attention rounds ----------------
            with tc.tile_pool(name="psrp", bufs=8, space="PSUM") as PSR:
                p23t = {}
                m23r = m23_sb[:].rearrange("p (jc k n) -> p jc k n",
                                           jc=JC, k=2)

                def build_head(h):
                    p23h = PP.tile([128, JC * 2 * N], BF16, tag="p23",
                                   name=f"p23_h{h}")
                    p23t[h] = p23h
                    p23hr = p23h[:].rearrange("p (jc k n) -> p jc k n",
                                              jc=JC, k=2)
                    for jc in range(JC):
                        idx = h * JC + jc
                        u_t = UP.tile([128, N], BF16, tag="u_t",
                                      name=f"u_{h}_{jc}")
                        nc.vector.tensor_scalar(
                            u_t[:], r_rep[:, h * N:(h + 1) * N],
                            ed_sb[:, h * JC + jc:h * JC + jc + 1],
                            ed2_sb[:, h * JC + jc:h * JC + jc + 1],
                            OP.mult, OP.max)
                        ubc = u_t[:].rearrange("p (k n) -> p k n", k=1) \
                                    .broadcast_to([128, 2, N])
                        if POOL_PU[idx]:
                            for k in range(2):
                                nc.gpsimd.tensor_tensor(
                                    p23hr[:, jc, k], u_t[:],
                                    m23r[:, jc, k], OP.mult)
                        else:
                            nc.vector.tensor_tensor(
                                p23hr[:, jc], ubc, m23r[:, jc], OP.mult)

                def mms_head(h, psa):
                    hh = h % 2
                    p23h = p23t[h]
                    for ic in range(IC):
                        for k in range(2):
                            off = hh * 256 + k * 128
                            for jc in range(JC):
                                nc.tensor.matmul(
                                    psa[ic][:, off:off + 97],
                                    p23h[:, jc * 1024 + k * 512 + ic * 128:
                                         jc * 1024 + k * 512 + (ic + 1) * 128],
                                    h23r[:, jc, h, k, 0:97],
                                    start=(jc == 0), stop=(jc == JC - 1))

                def evac_ic(rnd, psa, ic):
                    par = psa[ic][:].rearrange("p (s k o) -> p s k o",
                                               s=2, k=2, o=128)
                    dden = EV.tile([128, 4], F32, tag="dden",
                                   name=f"dd_{rnd}_{ic}")
                    rcol = EV.tile([128, 4], F32, tag="rcol",
                                   name=f"rc_{rnd}_{ic}")
                    t23 = EV.tile([128, 384], BF16, tag="t23",
                                  name=f"t23_{rnd}_{ic}")
                    ddenr = dden[:].rearrange("p (s k o) -> p s k o",
                                              s=2, k=2, o=1)
                    nc.vector.tensor_scalar(
                        ddenr, par[:, :, :, 96:97], EPS, 3.0,
                        OP.add, OP.mult)
                    nc.vector.reciprocal(rcol[:], dden[:])
                    t23r = t23[:].rearrange("p (s k e) -> p s k e",
                                            s=2, k=2, e=96)
                    if ic >= 2:
                        for s in range(2):
                            for k in range(2):
                                nc.scalar.activation(
                                    t23r[:, s, k, :], par[:, s, k, 0:96],
                                    AF.Copy,
                                    scale=rcol[:, 2 * s + k:2 * s + k + 1])
                    else:
                        rbc = rcol[:].rearrange("p (s k) -> p s k",
                                                s=2, k=2) \
                                     .broadcast_to([128, 2, 2, 96])
                        nc.vector.tensor_tensor(t23r, par[:, :, :, 0:96],
                                                rbc, OP.mult)
                    nc.vector.tensor_tensor(
                        preS[:, ic * D + rnd * 192:ic * D + rnd * 192 + 192]
                        .rearrange("p (s u e) -> p s u e", s=2, u=1, e=96),
                        t23r[:, :, 0:1, :], t23r[:, :, 1:2, :], OP.add)

                HD = 384

                def emit_ic(hf, ic):
                    lo = ic * D + hf * HD
                    pre = FP.tile([128, HD], BF16, tag="pre",
                                  name=f"pre_{ic}_{hf}")
                    nc.vector.tensor_tensor(pre[:], preS[:, lo:lo + HD],
                                            intra_bf[:, lo:lo + HD], OP.add)
                    if with_bias:
                        nc.vector.tensor_tensor(
                            pre[:], pre[:],
                            b3_sb[:, hf * HD:(hf + 1) * HD], OP.add)
                    e1 = FP.tile([128, HD], BF16, tag="e1",
                                 name=f"e1_{ic}_{hf}")
                    nc.scalar.activation(e1[:], pre[:], AF.Exp)
                    rl = FP.tile([128, HD], BF16, tag="rl",
                                 name=f"rl_{ic}_{hf}")
                    nc.vector.tensor_scalar(rl[:], pre[:], 0.0, None, OP.max)
                    # elu = min(e1 - 1, relu(pre))
                    elu = FP.tile([128, HD], BF16, tag="elu",
                                  name=f"elu_{ic}_{hf}")
                    nc.vector.scalar_tensor_tensor(
                        elu[:], e1[:], -1.0, rl[:], OP.add, OP.min)
                    # out = feat + (1+t)/2*(elu-feat) = feat + (d + t*d)/2
                    d_t = FP.tile([128, HD], BF16, tag="d_t",
                                  name=f"d_{ic}_{hf}")
                    nc.vector.tensor_tensor(d_t[:], elu[:],
                                            feat_sb[:, lo:lo + HD],
                                            OP.subtract)
                    gd = FP.tile([128, HD], BF16, tag="gd",
                                 name=f"gd_{ic}_{hf}")
                    nc.vector.tensor_tensor(gd[:], gate_bf[:, lo:lo + HD],
                                            d_t[:], OP.mult)
                    outf = FP.tile([128, HD], F32, tag="outf",
                                   name=f"of_{ic}_{hf}")
                    nc.gpsimd.tensor_tensor(outf[:], gd[:],
                                            feat_sb[:, lo:lo + HD], OP.add)
                    nc.sync.dma_start(
                        out[ic * 128:(ic + 1) * 128,
                            hf * HD:(hf + 1) * HD], outf[:])

                def alloc_psa(rnd):
                    return [PSR.tile([128, 512], F32, tag="psa",
                                     name=f"psa_r{rnd}_{i}")
                            for i in range(IC)]

                psas = {}
                for rnd in range(4):
                    h0 = rnd * 2
                    build_head(h0)
                    build_head(h0 + 1)
                    psas[rnd] = alloc_psa(rnd)
                    mms_head(h0, psas[rnd])
                    mms_head(h0 + 1, psas[rnd])
                    if rnd >= 1:
                        for ic in range(IC):
                            evac_ic(rnd - 1, psas[rnd - 1], ic)
                            if rnd == 2:
                                emit_ic(0, ic)
                for ic in range(IC):
                    evac_ic(3, psas[3], ic)
                    emit_ic(1, ic)

            FP.release()
            EV.release()
            PP.release()
            UP.release()

    nc.compile()
    return nc


def _prep_shared(W1, W2, W3, w_src, w_dst, H_w, H_b, b):
    f32 = np.float32
    BF = ml_dtypes.bfloat16
    F8 = ml_dtypes.float8_e4m3
    W1 = np.asarray(W1, f32)
    # [p, (h, dc, e)]
    W1p = np.ascontiguousarray(
        W1.reshape(H, DC, 128, E).transpose(2, 0, 1, 3)
        .reshape(128, H * DC * E)).astype(BF)
    W23 = np.stack([np.asarray(W2, f32).reshape(H, DC, 128, E),
                    np.asarray(W3, f32).reshape(H, DC, 128, E)], axis=2)
    W23f8 = np.ascontiguousarray(
        W23.transpose(3, 1, 0, 2, 4)
        .reshape(128, DC * H * 2 * E)).astype(F8)
    wsT = np.asarray(w_src, f32)[:, :, 0].T       # [96, H]
    wdT = np.asarray(w_dst, f32)[:, :, 0].T
    wsd_bf = np.ascontiguousarray(
        np.concatenate([wsT, wdT], axis=1)).astype(BF)       # [96, 16]
    wsr = np.ascontiguousarray(
        np.broadcast_to(wsT[:, :, None], (96, H, 128))
        .reshape(96, H * 128)).astype(BF)
    Hwt_f = np.ascontiguousarray(np.asarray(H_w, f32).T
                                 .reshape(DC, 128, D).transpose(1, 0, 2)
                                 .reshape(128, DC * D))
    Hbr = np.ascontiguousarray(np.asarray(H_b, f32).reshape(1, D)).astype(BF)
    shared = {
        "W1p": W1p, "W23f8": W23f8, "wsd": wsd_bf, "wsr": wsr, "Hb": Hbr,
        "eye128": np.eye(128).astype(BF),
        "ones_row": np.ones((1, 512), BF),
    }
    if GATE_FP8:
        shared["Hwt8"] = Hwt_f.astype(F8)
    else:
        shared["Hwt"] = Hwt_f.astype(BF)
    b = np.asarray(b, f32)
    with_bias = bool(np.any(b != 0))
    if with_bias:
        shared["b3row"] = np.ascontiguousarray(
            np.tile(b / 3.0, H).reshape(1, D)).astype(BF)
    return shared, with_bias


def _prep_core(feat, adjb, smb):
    f32 = np.float32
    BF = ml_dtypes.bfloat16
    F8 = ml_dtypes.float8_e4m3
    feat = np.asarray(feat, f32)
    feat_bf = np.ascontiguousarray(
        feat.reshape(IC, 128, D).transpose(1, 0, 2).reshape(128, IC * D)
    ).astype(BF)
    featT_f = np.ascontiguousarray(
        feat.T.reshape(DC, 128, N).transpose(1, 0, 2).reshape(128, DC * N))
    eye = np.eye(N, dtype=f32)
    m2 = smb.astype(f32) * (1.0 - eye)
    m3 = adjb.astype(f32) * (1.0 - smb.astype(f32))
    m23 = np.stack([m2.T.reshape(JC, 128, N), m3.T.reshape(JC, 128, N)],
                   axis=1)                        # [JC, 2, 128, N]
    m23T = np.ascontiguousarray(
        m23.transpose(2, 0, 1, 3).reshape(128, JC * 2 * N)).astype(BF)
    adjd3 = np.ascontiguousarray(
        (np.diagonal(adjb).astype(f32) / 3.0).reshape(IC, 128).T)
    return {"feat_bf": feat_bf,
            "featT": featT_f.astype(BF),
            "featT8": featT_f.astype(F8),
            "m23T": m23T, "adjd3": adjd3}


def kernel(feat_in, adj, relation, s_mask, W1, W2, W3, b, w_src, w_dst,
           H_w, H_b, **_unused):
    global _CACHED
    shared, with_bias = _prep_shared(W1, W2, W3, w_src, w_dst, H_w, H_b, b)
    if _CACHED is None or _CACHED[1] != with_bias:
        _CACHED = (build_kernel(with_bias), with_bias)
    nc = _CACHED[0]

    feat_in = np.asarray(feat_in, np.float32)
    adj = np.asarray(adj, np.int32)
    s_mask = np.asarray(s_mask, np.int32)
    in_maps = []
    for c in range(B):
        m = dict(shared)
        m.update(_prep_core(feat_in[c], adj[c], s_mask[c]))
        in_maps.append(m)
    res = run_bass_kernel_spmd(nc, in_maps, core_ids=list(range(B)))
    outp = np.stack([res.results[c]["out"] for c in range(B)], axis=0)
    return outp.astype(np.float32)



# revision 19
# speedup vs baseline: 1.3280x; 1.3280x over previous
"""Trainium2 Bass kernel for nn_EnhancedAttentionLayer (GAT-style masked attention).

Data-parallel over batch: B=8 batch elements -> 8 NeuronCores, one each.
Params replicated. No collectives.

Math (per batch element): h{1,2,3} = feat @ W{1,2,3}[h]; t1 = tanh(h1);
src/dst scores from t1; attn = leaky_relu(src_i + dst_j); two masked
softmaxes (m2 same-clause, m3 adj cross-clause) weight h2/h3; intra term
diag(adj)*h1; avg/3 (+bias) -> elu -> sigmoid(H feat)-gated residual.

Factorization: exp(leaky(z)) = exp(.2 z) * max(exp(.8 z), 1); exp(.2 src_i)
cancels in the masked softmax, so with r = exp(.8 src), ed = exp(dst),
ed2 = exp(.2 dst):  u[j,i] = max(r_i*ed_j, ed2_j)  (one fused 4x DVE op per
(h,jc)) and p_k = m_k * u.  Masks ship as {0, 1e30} so the product is
p = min(m, u): Pool prices "min" at the 0.60 default efficiency vs 0.42 for
Multiply; DVE is the same 2x TensorTensor either way.

Structure:
 - PE: fp8-DoubleRow h1 -> tanh -> scores, interleaved with one fp8
   [h1|h2|h3] GEMM (bf16 PSUM; h1 row-major feeds the intra term directly,
   so no transposes / h1T copies); fp8 gate GEMM (gate via tanh identity --
   Sigmoid lives in another activation table and would cost 2 table loads);
   97-wide attention-apply matmuls (built-in ones column = denominator)
   accumulating in bf16 PSUM, one [128,2048] tile per 2-head round.
 - Mask products split DVE/Pool per head; evac is round-batched (4 wide ops
   per round); emit chain is 2x-friendly: elu+1 = min(exp(pre), relu(pre)+1)
   and out = feat + gate*((elu+1) - (feat+1)).
 - PSUM: A[ps1|pss|psd|psb] -> B[psg|psa] so attention matmuls never wait
   on a pool transition; all DMAs on the sync queue in need-order; bf16 out.
"""

import numpy as np
import ml_dtypes

import concourse.bass as bass
import concourse.tile as tile
from concourse import bacc, mybir
from concourse.bass_utils import run_bass_kernel_spmd

F32 = mybir.dt.float32
BF16 = mybir.dt.bfloat16
FP8 = mybir.dt.float8e4
AF = mybir.ActivationFunctionType
OP = mybir.AluOpType
PM = mybir.MatmulPerfMode

B, N, D = 8, 512, 768
H, E = 8, 96
IC = N // 128
JC = N // 128
DC = D // 128
KT3 = 3
EPS = 1e-30
MBIG = 1.0
CH = 3 * E + 2      # 290 cols per head in h123 (h1 | h2 | one | h3 | one)
GW = 3 * E          # 288 cols per head in the W123 GEMM

# per-head: first POOL_JC[h] j-chunks of the mask product on DVE, rest Pool
POOL_JC = (3, 3, 3, 3, 3, 3, 3, 3)
# GEMM evacuation copy engine per (mc, quarter) index 0..15
COPY_ENG = ("act", "act", "dve", "act", "act", "dve", "act", "act",
            "dve", "act", "act", "dve", "act", "dve", "act", "dve")
OUTF_POOL = (0, 2)             # emit ics whose final add runs on Pool

DEBUG_DUMP = None   # None | "h123" | "p23h0" | "preS" | "r_ed"
_CACHED = None


def build_kernel(with_bias: bool):
    nc = bacc.Bacc("TRN2", target_bir_lowering=False, debug=False, num_devices=B)

    feat_bf = nc.dram_tensor("feat_bf", [128, IC * D], BF16, kind="ExternalInput").ap()
    featT8 = nc.dram_tensor("featT8", [128, DC * N], FP8, kind="ExternalInput").ap()
    m23T = nc.dram_tensor("m23T", [128, JC * 2 * N], BF16, kind="ExternalInput").ap()
    adjd3 = nc.dram_tensor("adjd3", [128, IC], F32, kind="ExternalInput").ap()
    W1p8 = nc.dram_tensor("W1p8", [128, H * DC * E], FP8, kind="ExternalInput").ap()
    W123f8 = nc.dram_tensor("W123f8", [128, DC * H * GW], FP8, kind="ExternalInput").ap()
    wsr = nc.dram_tensor("wsr", [96, H * 128], BF16, kind="ExternalInput").ap()
    wsd = nc.dram_tensor("wsd", [96, 16], BF16, kind="ExternalInput").ap()
    Hwt8 = nc.dram_tensor("Hwt8", [128, DC * D], FP8, kind="ExternalInput").ap()
    Hb = nc.dram_tensor("Hb", [1, D], BF16, kind="ExternalInput").ap()
    ones_row = nc.dram_tensor("ones_row", [1, 128], BF16, kind="ExternalInput").ap()
    if with_bias:
        b3row = nc.dram_tensor("b3row", [1, D], BF16, kind="ExternalInput").ap()
    out = nc.dram_tensor("out", [N, D], BF16, kind="ExternalOutput").ap()
    dbg = (nc.dram_tensor("dbg", [128, IC * H * CH], BF16,
                          kind="ExternalOutput").ap()
           if DEBUG_DUMP else None)

    with tile.TileContext(nc) as tc:
        with tc.tile_pool(name="persist", bufs=1) as P:
            adjd3_sb = P.tile([128, IC], F32, tag="adjd3_sb")
            Hb_sb = P.tile([1, D], BF16, tag="Hb_sb")
            onesr_sb = P.tile([1, 128], BF16, tag="onesr_sb")
            m23_sb = P.tile([128, JC * 2 * N], BF16, tag="m23_sb")     # 8K
            h123 = P.tile([128, IC * H * CH], BF16, tag="h123")        # 18.1K
            r_rep = P.tile([128, H * N], BF16, tag="r_rep")            # 8K
            ed_sb = P.tile([128, H * JC], F32, tag="ed_sb")
            ed2_sb = P.tile([128, H * JC], F32, tag="ed2_sb")
            tgate = P.tile([128, IC * D], BF16, tag="tgate")           # 6K
            gate_sb = P.tile([128, IC * D], BF16, tag="gate_sb")       # 6K
            preS = P.tile([128, IC * D], BF16, tag="preS")             # 6K
            intra_bf = P.tile([128, IC * D], BF16, tag="intra_bf")     # 6K
            feat_sb = P.tile([128, IC * D], BF16, tag="feat_sb")       # 6K
            f1_sb = P.tile([128, IC * D], BF16, tag="f1_sb")           # 6K
            b3_sb = P.tile([128, D], BF16, tag="b3_sb") if with_bias else None

            h123r = h123[:].rearrange("p (mc h c) -> p mc h c", mc=IC, h=H)
            m23r = m23_sb[:].rearrange("p (jc k n) -> p jc k n", jc=JC, k=2)

            UP = tc.alloc_tile_pool(name="upool", bufs=2)
            PP = tc.alloc_tile_pool(name="ppool", bufs=8)
            EV = tc.alloc_tile_pool(name="evpool", bufs=2)
            FP = tc.alloc_tile_pool(name="fpool", bufs=2)

            # ones columns of h123 are data-independent: set them first
            for mc in range(IC):
                nc.gpsimd.memset(h123r[:, mc, :, 2 * E:2 * E + 1], 1.0)
                nc.gpsimd.memset(h123r[:, mc, :, CH - 1:CH], 1.0)

            with tc.tile_pool(name="wpool", bufs=1) as WP:
                featT8_sb = WP.tile([128, DC * N], FP8, tag="featT8_sb")
                W1p_sb = WP.tile([128, H * DC * E], FP8, tag="W1p_sb")
                W123_sb = WP.tile([128, DC * H * GW], FP8, tag="W123_sb")
                Hwt_sb = WP.tile([128, DC * D], FP8, tag="Hwt_sb")
                wsr_sb = WP.tile([96, H * 128], BF16, tag="wsr_sb")
                wsd_sb = WP.tile([96, 16], BF16, tag="wsd_sb")
                TP = tc.alloc_tile_pool(name="tpool", bufs=1)
                t1T = TP.tile([96, H * N], BF16, tag="t1T")

                # ---- ALL DMAs on sync queue, in exact need-order ----
                nc.sync.dma_start(featT8_sb[:], featT8)
                nc.sync.dma_start(W1p_sb[:], W1p8)
                nc.sync.dma_start(wsd_sb[:], wsd)
                nc.sync.dma_start(wsr_sb[:], wsr)
                nc.sync.dma_start(W123_sb[:], W123f8)
                nc.sync.dma_start(m23_sb[:], m23T)
                nc.sync.dma_start(Hwt_sb[:], Hwt8)
                nc.sync.dma_start(Hb_sb[:], Hb)
                nc.sync.dma_start(onesr_sb[:], ones_row)
                nc.sync.dma_start(adjd3_sb[:], adjd3)
                nc.sync.dma_start(feat_sb[:], feat_bf)
                if with_bias:
                    b3d = WP.tile([1, D], BF16, tag="b3d")
                    nc.sync.dma_start(b3d[:], b3row)

                featT8r = featT8_sb[:].rearrange("p (kt n) -> p kt n", kt=DC)
                W1pr = W1p_sb[:].rearrange("p (h kt e) -> p h kt e",
                                           h=H, kt=DC)
                W123r = W123_sb[:].rearrange("p (kt c) -> p kt c", kt=DC)
                Hwtr = Hwt_sb[:].rearrange("p (kt f) -> p kt f", kt=DC)

                def build_head(h, p23t):
                    u4 = UP.tile([128, JC * N], BF16, tag="u4",
                                 name=f"u4_{h}")
                    for jc in range(JC):
                        g = h * JC + jc
                        nc.vector.tensor_scalar(
                            u4[:, jc * N:(jc + 1) * N],
                            r_rep[:, h * N:(h + 1) * N],
                            ed_sb[:, g:g + 1], ed2_sb[:, g:g + 1],
                            OP.mult, OP.max)
                    p23h = PP.tile([128, JC * 2 * N], BF16, tag="p23",
                                   name=f"p23_h{h}")
                    p23t[h] = p23h
                    u4r = u4[:].rearrange("p (jc n) -> p jc n", jc=JC)
                    dst = p23h[:].rearrange("p (jc k n) -> p jc k n",
                                            jc=JC, k=2)
                    pj = POOL_JC[h]
                    for k in range(2):
                        if pj < JC:
                            nc.vector.tensor_tensor(
                                dst[:, 0:pj, k, :], u4r[:, 0:pj, :],
                                m23r[:, 0:pj, k, :], OP.mult)
                            nc.gpsimd.tensor_tensor(
                                dst[:, pj:JC, k, :], u4r[:, pj:JC, :],
                                m23r[:, pj:JC, k, :], OP.mult)
                        else:
                            nc.vector.tensor_tensor(
                                dst[:, :, k, :], u4r,
                                m23r[:, :, k, :], OP.mult)

                p23t = {}

                def scores_pair(h0, PSS, PSD):
                    psd = PSD.tile([128, 8], F32, tag="psd",
                                   name=f"psd_{h0}")
                    for hh in range(2):
                        h = h0 + hh
                        pss = PSS.tile([128, 512], F32, tag="pss",
                                       name=f"pss_{h}")
                        nc.tensor.matmul(pss[:, :],
                                         wsr_sb[0:96, h * 128:(h + 1) * 128],
                                         t1T[0:96, h * N:(h + 1) * N],
                                         start=True, stop=True)
                        nc.scalar.activation(r_rep[:, h * N:(h + 1) * N],
                                             pss[:, :], AF.Exp, scale=0.8)
                        for jc in range(JC):
                            nc.tensor.matmul(
                                psd[:, hh * 4 + jc:hh * 4 + jc + 1],
                                t1T[0:96, h * N + jc * 128:
                                    h * N + (jc + 1) * 128],
                                wsd_sb[0:96, 8 + h:9 + h],
                                start=True, stop=True)
                    g0 = h0 * JC
                    nc.scalar.activation(ed_sb[:, g0:g0 + 8], psd[:, :],
                                         AF.Exp)
                    nc.scalar.activation(ed2_sb[:, g0:g0 + 8], psd[:, :],
                                         AF.Exp, scale=0.2)

                def gemm_mc(mc, PSB):
                    for q in range(4):
                        h0 = q * 2
                        # two heads per tile at bank-aligned offsets 0/512;
                        # each head's k3 accumulation chain is uninterrupted
                        psb = PSB.tile([128, 1024], F32, tag="psb",
                                       name=f"psb_{mc}_{q}")
                        for hh in range(2):
                            for k3 in range(KT3):
                                nc.tensor.matmul(
                                    psb[:, hh * 512:hh * 512 + GW],
                                    featT8r[:, 2 * k3:2 * k3 + 2,
                                            mc * 128:(mc + 1) * 128],
                                    W123r[:, 2 * k3:2 * k3 + 2,
                                          (h0 + hh) * GW:(h0 + hh + 1) * GW],
                                    start=(k3 == 0), stop=(k3 == KT3 - 1),
                                    perf_mode=PM.DoubleRow)
                        psbr = psb[:].rearrange("p (hh c) -> p hh c", hh=2)
                        dst1 = h123r[:, mc, h0:h0 + 2, 0:2 * E]
                        dst2 = h123r[:, mc, h0:h0 + 2, 2 * E + 1:CH - 1]
                        ceng = COPY_ENG[mc * 4 + q]
                        if ceng == "act":
                            nc.scalar.activation(dst1, psbr[:, :, 0:2 * E],
                                                 AF.Copy)
                            nc.scalar.activation(dst2, psbr[:, :, 2 * E:GW],
                                                 AF.Copy)
                        else:
                            nc.vector.tensor_copy(dst1, psbr[:, :, 0:2 * E])
                            nc.vector.tensor_copy(dst2, psbr[:, :, 2 * E:GW])
                    # intra term for this row chunk straight from h123
                    nc.vector.tensor_scalar(
                        intra_bf[:, mc * D:(mc + 1) * D]
                        .rearrange("p (h e) -> p h e", h=H),
                        h123r[:, mc, :, 0:E],
                        adjd3_sb[:, mc:mc + 1], None, OP.mult)
                    # f1 = feat + 1 (for the fused elu blend)
                    nc.vector.tensor_scalar(
                        f1_sb[:, mc * D:(mc + 1) * D],
                        feat_sb[:, mc * D:(mc + 1) * D], 1.0, None, OP.add)

                # ------- phase A: h1 fp8 + scores + [h1|h2|h3] GEMM ---------
                with (
                    tc.tile_pool(name="ps1p", bufs=2, space="PSUM") as PS1,
                    tc.tile_pool(name="pssp", bufs=1, space="PSUM") as PSS,
                    tc.tile_pool(name="psdp", bufs=1, space="PSUM") as PSD,
                    tc.tile_pool(name="psbp", bufs=2, space="PSUM") as PSB,
                ):
                    for h in range(H):
                        ps1 = PS1.tile([96, 512], F32, tag="ps1",
                                       name=f"ps1_{h}")
                        for k3 in range(KT3):
                            nc.tensor.matmul(
                                ps1[:, :],
                                W1pr[:, h, 2 * k3:2 * k3 + 2, :],
                                featT8r[:, 2 * k3:2 * k3 + 2, :],
                                start=(k3 == 0), stop=(k3 == KT3 - 1),
                                perf_mode=PM.DoubleRow)
                        nc.scalar.activation(t1T[0:96, h * N:(h + 1) * N],
                                             ps1[:, :], AF.Tanh)
                        if h % 2 == 1:
                            scores_pair(h - 1, PSS, PSD)
                            build_head(h - 1, p23t)
                            build_head(h, p23t)
                            gemm_mc(h // 2, PSB)

                    if with_bias:
                        psbb = PSB.tile([128, 2 * GW], F32, tag="psb",
                                        name="psb_bias")
                        nc.tensor.matmul(psbb[:, 0:512],
                                         onesr_sb[0:1, 0:128],
                                         b3d[0:1, 0:512], start=True,
                                         stop=True)
                        nc.tensor.matmul(psbb[:, 512:576],
                                         onesr_sb[0:1, 0:128],
                                         b3d[0:1, 512:576], start=True,
                                         stop=True)
                        nc.vector.tensor_copy(b3_sb[:, 0:576],
                                              psbb[:, 0:576])
                        psbc = PSB.tile([128, 2 * GW], F32, tag="psb",
                                        name="psb_bias2")
                        nc.tensor.matmul(psbc[:, 0:192],
                                         onesr_sb[0:1, 0:128],
                                         b3d[0:1, 576:768], start=True,
                                         stop=True)
                        nc.vector.tensor_copy(b3_sb[:, 576:768],
                                              psbc[:, 0:192])
                    TP.release()

                # ------- phase B: gate GEMM + attention rounds --------------
                with (
                    tc.tile_pool(name="psgp", bufs=1, space="PSUM") as PSG,
                    tc.tile_pool(name="psrp", bufs=3, space="PSUM") as PSR,
                ):
                    for mc in range(IC):
                        psg = PSG.tile([128, D], F32, tag="psg",
                                       name=f"psg_{mc}")
                        for half, lo, hi in ((0, 0, 512), (1, 512, 768)):
                            for k3 in range(KT3):
                                nc.tensor.matmul(
                                    psg[:, lo:hi],
                                    featT8r[:, 2 * k3:2 * k3 + 2,
                                            mc * 128:(mc + 1) * 128],
                                    Hwtr[:, 2 * k3:2 * k3 + 2, lo:hi],
                                    start=(k3 == 0), stop=False,
                                    perf_mode=PM.DoubleRow)
                            nc.tensor.matmul(psg[:, lo:hi],
                                             onesr_sb[0:1, 0:128],
                                             Hb_sb[0:1, lo:hi], start=False,
                                             stop=True)
                        # sigmoid(x) = (1 + tanh(x/2))/2 -- same act table
                        nc.scalar.activation(tgate[:, mc * D:(mc + 1) * D],
                                             psg[:, 0:768], AF.Tanh,
                                             scale=0.5)
                        nc.vector.tensor_scalar(
                            gate_sb[:, mc * D:(mc + 1) * D],
                            tgate[:, mc * D:(mc + 1) * D],
                            0.5, 0.5, OP.mult, OP.add)

                    def mms_head(h, psa01, psa23):
                        hh = h % 2
                        p23h = p23t[h]
                        for ic in range(IC):
                            psa = psa01 if ic < 2 else psa23
                            for k in range(2):
                                off = (ic % 2) * 512 + hh * 256 + k * 128
                                rlo = 2 * E + k * (E + 1)
                                for jc in range(JC):
                                    nc.tensor.matmul(
                                        psa[:, off:off + E + 1],
                                        p23h[:, jc * 1024 + k * 512
                                             + ic * 128:
                                             jc * 1024 + k * 512
                                             + (ic + 1) * 128],
                                        h123r[:, jc, h, rlo - E:rlo + 1],
                                        start=(jc == 0),
                                        stop=(jc == JC - 1))

                    def evac_half(rnd, half, psa):
                        # par grouped as [p, g=(i,s,k), 128]: 2 free dims
                        par = psa[:].rearrange("p (g o) -> p g o", g=8)
                        dden = EV.tile([128, 8], F32, tag="dden",
                                       name=f"dd_{rnd}_{half}")
                        rcol = EV.tile([128, 8], F32, tag="rcol",
                                       name=f"rc_{rnd}_{half}")
                        t23 = EV.tile([128, 2 * 4 * E], BF16, tag="t23",
                                      name=f"t23_{rnd}_{half}")
                        ddenr = dden[:].rearrange("p (g o) -> p g o", g=8)
                        nc.scalar.activation(ddenr, par[:, :, E:E + 1],
                                             AF.Copy, bias=3.0 * EPS,
                                             scale=3.0)
                        nc.vector.reciprocal(rcol[:], dden[:])
                        t23g = t23[:].rearrange("p (g e) -> p g e", g=8)
                        rbc = ddenr and rcol[:].rearrange(
                            "p (g o) -> p g o", g=8).broadcast_to([128, 8, E])
                        nc.vector.tensor_tensor(t23g, par[:, :, 0:E],
                                                rbc, OP.mult)
                        t23r = t23[:].rearrange("p (i s k e) -> p i s k e",
                                                i=2, s=2, k=2, e=E)
                        nc.vector.tensor_tensor(
                            preS[:].rearrange("p (i r s e) -> p i r s e",
                                              i=IC, r=4, s=2)
                            [:, 2 * half:2 * half + 2, rnd],
                            t23r[:, :, :, 0, :], t23r[:, :, :, 1, :],
                            OP.add)

                    HD = 384

                    def emit_ic(hf, ic):
                        lo = ic * D + hf * HD
                        pre = FP.tile([128, HD], BF16, tag="pre",
                                      name=f"pre_{ic}_{hf}")
                        nc.vector.tensor_tensor(pre[:], preS[:, lo:lo + HD],
                                                intra_bf[:, lo:lo + HD],
                                                OP.add)
                        if with_bias:
                            nc.vector.tensor_tensor(
                                pre[:], pre[:],
                                b3_sb[:, hf * HD:(hf + 1) * HD], OP.add)
                        e1 = FP.tile([128, HD], BF16, tag="e1",
                                     name=f"e1_{ic}_{hf}")
                        nc.scalar.activation(e1[:], pre[:], AF.Exp)
                        rl1 = FP.tile([128, HD], BF16, tag="rl1",
                                      name=f"rl1_{ic}_{hf}")
                        nc.vector.tensor_scalar(rl1[:], pre[:], 0.0, 1.0,
                                                OP.max, OP.add)
                        m1 = FP.tile([128, HD], BF16, tag="m1",
                                     name=f"m1_{ic}_{hf}")
                        nc.vector.tensor_tensor(m1[:], e1[:], rl1[:], OP.min)
                        d_t = FP.tile([128, HD], BF16, tag="d_t",
                                      name=f"d_{ic}_{hf}")
                        nc.vector.tensor_tensor(d_t[:], m1[:],
                                                f1_sb[:, lo:lo + HD],
                                                OP.subtract)
                        gd = FP.tile([128, HD], BF16, tag="gd",
                                     name=f"gd_{ic}_{hf}")
                        nc.vector.tensor_tensor(gd[:], gate_sb[:, lo:lo + HD],
                                                d_t[:], OP.mult)
                        outf = FP.tile([128, HD], BF16, tag="outf",
                                       name=f"of_{ic}_{hf}")
                        if ic in OUTF_POOL:
                            nc.gpsimd.tensor_tensor(outf[:], gd[:],
                                                    feat_sb[:, lo:lo + HD],
                                                    OP.add)
                        else:
                            nc.vector.tensor_tensor(outf[:], gd[:],
                                                    feat_sb[:, lo:lo + HD],
                                                    OP.add)
                        nc.sync.dma_start(
                            out[ic * 128:(ic + 1) * 128,
                                hf * HD:(hf + 1) * HD], outf[:])

                    if DEBUG_DUMP == "h123":
                        nc.sync.dma_start(dbg[:, 0:IC * H * CH], h123[:])
                    elif DEBUG_DUMP == "p23h0":
                        nc.sync.dma_start(dbg[:, 0:JC * 2 * N],
                                          p23t[0][:])
                        nc.sync.dma_start(dbg[:, JC * 2 * N:2 * JC * 2 * N],
                                          p23t[1][:])
                    elif DEBUG_DUMP == "r_ed":
                        nc.sync.dma_start(dbg[:, 0:H * N], r_rep[:])
                        nc.sync.dma_start(dbg[:, H * N:H * N + 32],
                                          ed_sb[:])
                        nc.sync.dma_start(dbg[:, H * N + 32:H * N + 64],
                                          ed2_sb[:])
                    psas = {}
                    for rnd in range(4):
                        h0 = rnd * 2
                        psas[rnd] = (
                            PSR.tile([128, 1024], F32, tag="psa",
                                     name=f"psa_r{rnd}_01"),
                            PSR.tile([128, 1024], F32, tag="psa",
                                     name=f"psa_r{rnd}_23"))
                        mms_head(h0, *psas[rnd])
                        mms_head(h0 + 1, *psas[rnd])
                        if rnd >= 1:
                            evac_half(rnd - 1, 0, psas[rnd - 1][0])
                            evac_half(rnd - 1, 1, psas[rnd - 1][1])
                            if rnd == 2:
                                for ic in range(IC):
                                    emit_ic(0, ic)
                    evac_half(3, 0, psas[3][0])
                    evac_half(3, 1, psas[3][1])
                    if DEBUG_DUMP == "preS":
                        nc.sync.dma_start(dbg[:, 0:IC * D], preS[:])
                        nc.sync.dma_start(dbg[:, IC * D:2 * IC * D],
                                          intra_bf[:])
                    for ic in range(IC):
                        emit_ic(1, ic)

            FP.release()
            EV.release()
            PP.release()
            UP.release()

    nc.compile()
    return nc


def _prep_shared(W1, W2, W3, w_src, w_dst, H_w, H_b, b):
    f32 = np.float32
    BF = ml_dtypes.bfloat16
    F8 = ml_dtypes.float8_e4m3
    W1 = np.asarray(W1, f32)
    # W1p8: [p, (h, kt, e)], d = kt*128 + p
    W1p8 = np.ascontiguousarray(
        W1.reshape(H, DC, 128, E).transpose(2, 0, 1, 3)
        .reshape(128, H * DC * E)).astype(F8)
    # W123: [p, (kt, h, 3*96)]
    W123 = np.concatenate(
        [W1, np.asarray(W2, f32), np.asarray(W3, f32)], axis=2)  # [H,768,288]
    W123f8 = np.ascontiguousarray(
        W123.reshape(H, DC, 128, GW).transpose(2, 1, 0, 3)
        .reshape(128, DC * H * GW)).astype(F8)
    wsT = np.asarray(w_src, f32)[:, :, 0].T       # [96, H]
    wdT = np.asarray(w_dst, f32)[:, :, 0].T
    wsd_bf = np.ascontiguousarray(
        np.concatenate([wsT, wdT], axis=1)).astype(BF)       # [96, 16]
    wsr = np.ascontiguousarray(
        np.broadcast_to(wsT[:, :, None], (96, H, 128))
        .reshape(96, H * 128)).astype(BF)
    Hwt8 = np.ascontiguousarray(np.asarray(H_w, f32).T
                                .reshape(DC, 128, D).transpose(1, 0, 2)
                                .reshape(128, DC * D)).astype(F8)
    Hbr = np.ascontiguousarray(np.asarray(H_b, f32).reshape(1, D)).astype(BF)
    shared = {
        "W1p8": W1p8, "W123f8": W123f8, "wsd": wsd_bf, "wsr": wsr,
        "Hwt8": Hwt8, "Hb": Hbr,
        "ones_row": np.ones((1, 128), BF),
    }
    b = np.asarray(b, f32)
    with_bias = bool(np.any(b != 0))
    if with_bias:
        shared["b3row"] = np.ascontiguousarray(
            np.tile(b / 3.0, H).reshape(1, D)).astype(BF)
    return shared, with_bias


def _prep_core(feat, adjb, smb):
    f32 = np.float32
    BF = ml_dtypes.bfloat16
    F8 = ml_dtypes.float8_e4m3
    feat = np.asarray(feat, f32)
    feat_bf = np.ascontiguousarray(
        feat.reshape(IC, 128, D).transpose(1, 0, 2).reshape(128, IC * D)
    ).astype(BF)
    featT_f = np.ascontiguousarray(
        feat.T.reshape(DC, 128, N).transpose(1, 0, 2).reshape(128, DC * N))
    eye = np.eye(N, dtype=f32)
    m2 = smb.astype(f32) * (1.0 - eye)
    m3 = adjb.astype(f32) * (1.0 - smb.astype(f32))
    m23 = np.stack([m2.T.reshape(JC, 128, N), m3.T.reshape(JC, 128, N)],
                   axis=1) * MBIG                 # [JC, 2, 128, N]
    m23T = np.ascontiguousarray(
        m23.transpose(2, 0, 1, 3).reshape(128, JC * 2 * N)).astype(BF)
    adjd3 = np.ascontiguousarray(
        (np.diagonal(adjb).astype(f32) / 3.0).reshape(IC, 128).T)
    return {"feat_bf": feat_bf,
            "featT8": featT_f.astype(F8),
            "m23T": m23T, "adjd3": adjd3}


def kernel(feat_in, adj, relation, s_mask, W1, W2, W3, b, w_src, w_dst,
           H_w, H_b, **_unused):
    global _CACHED
    shared, with_bias = _prep_shared(W1, W2, W3, w_src, w_dst, H_w, H_b, b)
    if _CACHED is None or _CACHED[1] != with_bias:
        _CACHED = (build_kernel(with_bias), with_bias)
    nc = _CACHED[0]

    feat_in = np.asarray(feat_in, np.float32)
    adj = np.asarray(adj, np.int32)
    s_mask = np.asarray(s_mask, np.int32)
    in_maps = []
    for c in range(B):
        m = dict(shared)
        m.update(_prep_core(feat_in[c], adj[c], s_mask[c]))
        in_maps.append(m)
    res = run_bass_kernel_spmd(nc, in_maps, core_ids=list(range(B)))
    outp = np.stack([res.results[c]["out"] for c in range(B)], axis=0)
    return outp.astype(np.float32)
